# revision 32
# baseline (speedup 1.0000x reference)
"""Bass/Trainium2 kernel for nn_NeuroBiMambaBlock.

Sharding: 8 cores = 4 samples x 2 directions (fwd/bwd mamba). Every core
runs an identical SPMD program on its own data: bwd cores receive the
time-flipped sample and the b_* weight set, so their mamba scan is
forward-in-layout. The outer (shared) stage is replicated per pair; its
causal conv becomes anti-causal on flipped cores, handled by a 7-tap
"wide" conv whose taps the host builds per direction. Each core returns
a partial output (its direction's contribution through the final
projection); the host sums the pair, un-flips the bwd part, and adds the
residual.

Layout on device: [feature -> partitions, time -> free]. The selective
scan runs as one tensor_tensor_scan per 128-channel block, with the 16
states per channel packed as 16 segments of (2 boundary cols + T token
cols); boundary col1 injects the carried state (a=0 there resets).

Engine split: depthwise convs run as diagonal matmuls on the PE; silu /
sigmoid / ln / PSUM->SBUF moves run on the Act engine; per-state decay
powers, dt*u*B and h*C products are batched into single strided
tensor ops with broadcast APs, distributed between DVE and Pool.

Four-stage software pipeline per 256-token tile — A: LN + outer in-proj
(PE/Act); B1: convs, inner proj, dt, B/C broadcast (PE/Act); B2: decay
powers, dBu, scan, h*C, tree-sum (DVE/Pool); C: out-proj + gating +
final projection + DMA. Stage k of tile i overlaps stage k-1 of tile
i+1, so Act's broadcast production always runs a full stage ahead of
the DVE consumer and the scan section pipelines cleanly.
"""

import numpy as np

B, L, DM = 4, 4096, 256
DH = 512
N = 16
KC = 4
R = 32
EPS = 1e-5
T = 256                  # tokens per pipeline tile
NT = L // T              # tiles
SEG = T + 2              # scan segment length (2 boundary cols + T tokens)
NDB = DH // 128          # 4 channel blocks
NTB = T // 128           # token blocks per tile

# engine assignment knobs (db indices whose ops run on Pool/gpsimd)
POW_POOL = frozenset()
DBU_POOL = frozenset({1, 2, 3})
HC_POOL = frozenset({2})
TREE_POOL = frozenset({2, 3})
BCALL_DVE = 2            # of 16 bcast copies, this many go to DVE (rest Act)

_CACHE = {}


def build_program(Lx=L):
    import concourse.bass as bass
    import concourse.bacc as bacc
    import concourse.tile as tile
    import concourse.mybir as mybir
    from contextlib import ExitStack

    f32 = mybir.dt.float32
    bf16 = mybir.dt.bfloat16
    AF = mybir.ActivationFunctionType
    OP = mybir.AluOpType
    AX = mybir.AxisListType

    nt = Lx // T
    nc = bacc.Bacc("TRN2", target_bir_lowering=False, debug=False)

    x_in = nc.declare_dram_parameter("x_in", [Lx, DM], bf16, isOutput=False)
    w_in_T = nc.declare_dram_parameter("w_in_T", [DM, 2 * DH], bf16, isOutput=False)
    gate_bias = nc.declare_dram_parameter("gate_bias", [DH, 1], f32, isOutput=False)
    w7d = nc.declare_dram_parameter("w7", [DH, 7], f32, isOutput=False)
    conv_bd = nc.declare_dram_parameter("conv_b", [DH, 1], f32, isOutput=False)
    m_in_T = nc.declare_dram_parameter("m_in_T", [DH, 2 * DH], bf16, isOutput=False)
    m_conv_wd = nc.declare_dram_parameter("m_conv_w", [DH, KC], f32, isOutput=False)
    m_conv_bd = nc.declare_dram_parameter("m_conv_b", [DH, 1], f32, isOutput=False)
    m_xproj_T = nc.declare_dram_parameter("m_xproj_T", [DH, R + 2 * N], bf16, isOutput=False)
    m_dt_wT = nc.declare_dram_parameter("m_dt_wT", [R, DH], bf16, isOutput=False)
    m_dt_bd = nc.declare_dram_parameter("m_dt_b", [DH, 1], f32, isOutput=False)
    m_out_T2 = nc.declare_dram_parameter("m_out_T2", [DH, DH], bf16, isOutput=False)
    m_Dd = nc.declare_dram_parameter("m_D", [DH, 1], f32, isOutput=False)
    w_out_sl_T = nc.declare_dram_parameter("w_out_sl_T", [DH, DM], bf16, isOutput=False)
    part = nc.declare_dram_parameter("part", [Lx, DM], bf16, isOutput=True)

    with tile.TileContext(nc) as tc, ExitStack() as ctx:
        wpool = ctx.enter_context(tc.tile_pool(name="weights", bufs=1))
        psum = ctx.enter_context(tc.tile_pool(name="psum", bufs=2, space="PSUM"))
        pconv = ctx.enter_context(tc.tile_pool(name="pconv", bufs=2, space="PSUM"))
        pbc = ctx.enter_context(tc.tile_pool(name="pbc", bufs=2, space="PSUM"))
        ptr = ctx.enter_context(tc.tile_pool(name="ptr", bufs=2, space="PSUM"))
        pp1 = ctx.enter_context(tc.tile_pool(name="pipe1", bufs=2))
        pp2 = ctx.enter_context(tc.tile_pool(name="pipe2", bufs=2))
        cinp = ctx.enter_context(tc.tile_pool(name="cin", bufs=2))
        xinp = ctx.enter_context(tc.tile_pool(name="xin", bufs=2))
        spool = ctx.enter_context(tc.tile_pool(name="scan", bufs=1))
        bcp = ctx.enter_context(tc.tile_pool(name="bcall", bufs=1))
        stp = ctx.enter_context(tc.tile_pool(name="state", bufs=2))
        smalls = ctx.enter_context(tc.tile_pool(name="smalls", bufs=2))

        # ---- weights to SBUF ----
        winT = []
        for kb in range(DM // 128):
            t = wpool.tile([128, 2 * DH], bf16, tag=f"winT{kb}", name=f"winT{kb}")
            nc.sync.dma_start(t[:], w_in_T[kb * 128:(kb + 1) * 128, :])
            winT.append(t)
        minT = []
        for kb in range(NDB):
            t = wpool.tile([128, 2 * DH], bf16, tag=f"minT{kb}", name=f"minT{kb}")
            nc.sync.dma_start(t[:], m_in_T[kb * 128:(kb + 1) * 128, :])
            minT.append(t)
        mxpT = []
        for kb in range(NDB):
            t = wpool.tile([128, R + 2 * N], bf16, tag=f"mxpT{kb}", name=f"mxpT{kb}")
            nc.sync.dma_start(t[:], m_xproj_T[kb * 128:(kb + 1) * 128, :])
            mxpT.append(t)
        mdtT = wpool.tile([R, DH], bf16)
        nc.sync.dma_start(mdtT[:], m_dt_wT[:])
        moT2 = []
        for kb in range(DH // 128):
            t = wpool.tile([128, DH], bf16, tag=f"moT2_{kb}", name=f"moT2_{kb}")
            nc.sync.dma_start(t[:], m_out_T2[kb * 128:(kb + 1) * 128, :])
            moT2.append(t)
        woT = []
        for kb in range(NDB):
            t = wpool.tile([128, DM], bf16, tag=f"woT{kb}", name=f"woT{kb}")
            nc.sync.dma_start(t[:], w_out_sl_T[kb * 128:(kb + 1) * 128, :])
            woT.append(t)

        _cv = [0]
        def colvec(dram):
            out = []
            for db in range(NDB):
                _cv[0] += 1
                t = wpool.tile([128, 1], f32, tag=f"cv{_cv[0]}", name=f"cv{_cv[0]}")
                nc.sync.dma_start(t[:], dram[db * 128:(db + 1) * 128, :])
                out.append(t)
            return out

        mD = colvec(m_Dd)
        gbias = colvec(gate_bias)
        cbias = colvec(conv_bd)
        mcbias = colvec(m_conv_bd)
        mdtb = colvec(m_dt_bd)
        w7c, mcw = [], []
        for db in range(NDB):
            t = wpool.tile([128, 7], f32, tag=f"w7c{db}", name=f"w7c{db}")
            nc.sync.dma_start(t[:], w7d[db * 128:(db + 1) * 128, :])
            w7c.append(t)
            t2 = wpool.tile([128, KC], f32, tag=f"mcw{db}", name=f"mcw{db}")
            nc.sync.dma_start(t2[:], m_conv_wd[db * 128:(db + 1) * 128, :])
            mcw.append(t2)

        # identity for PE transposes + diag conv weights
        idf = wpool.tile([128, 128], f32)
        pidx = wpool.tile([128, 1], f32)
        nc.gpsimd.iota(idf[:], [[1, 128]], channel_multiplier=0,
                       allow_small_or_imprecise_dtypes=True)
        nc.gpsimd.iota(pidx[:], [[0, 1]], channel_multiplier=1,
                       allow_small_or_imprecise_dtypes=True)
        eqf = wpool.tile([128, 128], f32)
        nc.vector.tensor_scalar(eqf[:], idf[:], pidx[:], None, OP.is_equal)
        ident = wpool.tile([128, 128], bf16)
        nc.vector.tensor_copy(ident[:], eqf[:])
        dg_o = []          # outer conv diag taps [db][k]
        for db in range(NDB):
            row = []
            for k in range(7):
                dg = wpool.tile([128, 128], bf16, tag=f"dgo{db}_{k}", name=f"dgo{db}_{k}")
                nc.vector.tensor_scalar(dg[:], eqf[:], w7c[db][:, k:k + 1], None, OP.mult)
                row.append(dg)
            dg_o.append(row)
        dg_i = []          # inner conv diag taps [db][k]
        for db in range(NDB):
            row = []
            for k in range(KC):
                dg = wpool.tile([128, 128], bf16, tag=f"dgi{db}_{k}", name=f"dgi{db}_{k}")
                nc.vector.tensor_scalar(dg[:], eqf[:], mcw[db][:, k:k + 1], None, OP.mult)
                row.append(dg)
            dg_i.append(row)

        # selector [16, 16*128]: sel[k, n*128+m] = (k == n), for PE row-broadcast
        sel = wpool.tile([16, N * 128], bf16)
        nc.gpsimd.iota(sel[:], [[1, N], [0, 128]], channel_multiplier=0,
                       allow_small_or_imprecise_dtypes=True)
        pidx16 = wpool.tile([16, 1], f32)
        nc.gpsimd.iota(pidx16[:], [[0, 1]], channel_multiplier=1,
                       allow_small_or_imprecise_dtypes=True)
        nc.vector.tensor_scalar(sel[:], sel[:], pidx16[:], None, OP.is_equal)



        cin_prev = [None] * NDB
        g1_hist, sg_hist, hd_hist = {}, {}, {}
        xin_prev = [None] * NDB
        st_prev = [None] * NDB     # carried scan states [128,16] per dblk

        def seg3(ap):
            return ap[:].rearrange("p (n c) -> p n c", c=SEG)

        def run_head(j, cin_j):
            """Stage B1 of tile j: convs, projections, dt, bcall (PE+Act)."""
            # ---- outer wide conv (7 taps) on PE + silu on Act ----
            actT = [pp1.tile([128, T], bf16, tag=f"actT{db}", name=f"actT{db}") for db in range(NDB)]
            for db in range(NDB):
                pc = pconv.tile([128, T], f32, tag="cv")
                for k in range(7):
                    nc.tensor.matmul(pc[:], dg_o[db][k][:], cin_j[db][:, 3 + k:3 + k + T],
                                     start=(k == 0), stop=(k == 6))
                nc.scalar.activation(actT[db][:], pc[:], AF.Silu, bias=cbias[db][:])

            # ---- inner in-proj ----
            xin = [xinp.tile([128, T + 3], bf16, tag=f"xin{db}", name=f"xin{db}") for db in range(NDB)]
            szT = [pp1.tile([128, T], bf16, tag=f"szT{db}", name=f"szT{db}", bufs=3) for db in range(NDB)]
            zps = []
            for mb in range(2 * DH // 128):
                pt = psum.tile([128, T], f32, tag="mm")
                for kb in range(NDB):
                    nc.tensor.matmul(pt[:], minT[kb][:, mb * 128:(mb + 1) * 128],
                                     actT[kb][:], start=(kb == 0), stop=(kb == NDB - 1))
                if mb < NDB:
                    nc.scalar.copy(xin[mb][:, 3:3 + T], pt[:])
                else:
                    zps.append(pt)
            for db in range(NDB):
                if j == 0:
                    nc.gpsimd.memset(xin[db][:, 0:3], 0.0)
                else:
                    nc.vector.tensor_copy(xin[db][:, 0:3], xin_prev[db][:, T:T + 3])
                xin_prev[db] = xin[db]

            # ---- inner causal conv (4 taps) on PE + silu on Act ----
            uT = [pp1.tile([128, T], bf16, tag=f"uT{db}", name=f"uT{db}", bufs=3) for db in range(NDB)]
            for db in range(NDB):
                pc = pconv.tile([128, T], f32, tag="cv")
                for k in range(KC):
                    nc.tensor.matmul(pc[:], dg_i[db][k][:], xin[db][:, k:k + T],
                                     start=(k == 0), stop=(k == KC - 1))
                nc.scalar.activation(uT[db][:], pc[:], AF.Silu, bias=mcbias[db][:])

            # ---- xproj (dt-rank rows / combined B+C rows) ----
            pxd = psum.tile([R, T], f32, tag="mm")
            for kb in range(NDB):
                nc.tensor.matmul(pxd[:], mxpT[kb][:, 0:R], uT[kb][:],
                                 start=(kb == 0), stop=(kb == NDB - 1))
            pxb = psum.tile([N, T], f32, tag="mm")
            for kb in range(NDB):
                nc.tensor.matmul(pxb[:], mxpT[kb][:, R:R + N], uT[kb][:],
                                 start=(kb == 0), stop=(kb == NDB - 1))
            pxc = psum.tile([N, T], f32, tag="mm")
            for kb in range(NDB):
                nc.tensor.matmul(pxc[:], mxpT[kb][:, R + N:R + 2 * N], uT[kb][:],
                                 start=(kb == 0), stop=(kb == NDB - 1))
            xdbl = pp1.tile([R, T], bf16, tag="xdbl")
            nc.scalar.copy(xdbl[:], pxd[:])
            xdBC = pp1.tile([N, 2 * T], bf16, tag="xdBC")
            nc.scalar.copy(xdBC[:, 0:T], pxb[:])
            nc.scalar.copy(xdBC[:, T:2 * T], pxc[:])

            for db in range(NDB):
                nc.scalar.activation(szT[db][:], zps[db][:], AF.Silu)

            # ---- dt: rf = sigmoid(-(proj+b)) = exp(-dt); mdt = ln(rf) = -dt ----
            rT = [pp1.tile([128, T], bf16, tag=f"rT{db}", name=f"rT{db}") for db in range(NDB)]
            mdt = [pp1.tile([128, T], bf16, tag=f"mdt{db}", name=f"mdt{db}") for db in range(NDB)]
            for db in range(NDB):
                pt = psum.tile([128, T], f32, tag="mm")
                nc.tensor.matmul(pt[:], mdtT[:, db * 128:(db + 1) * 128],
                                 xdbl[:], start=True, stop=True)
                nc.scalar.activation(rT[db][:], pt[:], AF.Sigmoid, scale=-1.0,
                                     bias=mdtb[db][:])
            for db in range(NDB):
                nc.scalar.activation(mdt[db][:], rT[db][:], AF.Ln)

            # ---- broadcast B,C rows to 128 partitions: bcall[n] = [B_n | C_n] ----
            bcall = bcp.tile([128, N * 2 * T], bf16, tag="bcall")
            for n in range(N):
                pb = pbc.tile([128, 2 * T], f32, tag="bc")
                nc.tensor.matmul(pb[:], sel[:, n * 128:(n + 1) * 128], xdBC[:],
                                 start=True, stop=True)
                if n < BCALL_DVE:
                    nc.vector.tensor_copy(bcall[:, n * 2 * T:(n + 1) * 2 * T], pb[:])
                else:
                    nc.scalar.copy(bcall[:, n * 2 * T:(n + 1) * 2 * T], pb[:])
            return dict(bcall=bcall, rT=rT, mdt=mdt, uT=uT, szT=szT)

        def run_ssm(j, hd):
            """Stage B2 of tile j: powers, dBu, scan, hC, tree (DVE+Pool)."""
            bcall, rT, mdt, uT, szT = hd["bcall"], hd["rT"], hd["mdt"], hd["uT"], hd["szT"]
            bc3 = bcall[:].rearrange("p (n c) -> p n c", c=2 * T)
            dtuT = [pp1.tile([128, T], bf16, tag=f"dtuT{db}", name=f"dtuT{db}") for db in range(NDB)]
            for db in range(NDB):
                nc.vector.tensor_tensor(out=dtuT[db][:], in0=mdt[db][:],
                                        in1=uT[db][:], op=OP.mult)

            # ---- per channel block: powers, dBu, scan, hC, tree ----
            # Emitted stage-parallel (all dbs per stage) so independent db
            # work sits adjacent in each engine queue and overlaps.
            g1 = [pp1.tile([128, T], bf16, tag=f"g1{db}", name=f"g1{db}") for db in range(NDB)]
            ats, hts = [], []
            for db in range(NDB):
                vp = nc.gpsimd if db in POW_POOL else nc.vector
                # a_t: seg n tokens = r^(n+1); boundary cols 0 (via propagation)
                a_t = spool.tile([128, N * SEG], bf16, tag=f"a{db}",
                                 bufs=2 if db in TREE_POOL else 1)
                a3 = seg3(a_t)
                if j < 2:
                    nc.vector.memset(a3[:, 0:2, 0:2], 0.0)
                nc.vector.tensor_copy(a_t[:, 2:2 + T], rT[db][:])
                vp.tensor_tensor(out=a_t[:, SEG + 2:SEG + 2 + T], in0=rT[db][:],
                                 in1=rT[db][:], op=OP.mult)
                for (lo, hi) in ((2, 4), (4, 8), (8, 16)):
                    seg_hi = a_t[:, (lo - 1) * SEG:lo * SEG].rearrange(
                        "p (o c) -> p o c", o=1)
                    vp.tensor_tensor(out=a3[:, lo:hi, :], in0=a3[:, 0:hi - lo, :],
                                     in1=seg_hi.broadcast_to([128, hi - lo, SEG]),
                                     op=OP.mult)
                ats.append(a_t)
            dbs_ = [None] * NDB
            for db in sorted(range(NDB), key=lambda d: (d not in DBU_POOL, d)):
                vd = nc.gpsimd if db in DBU_POOL else nc.vector
                # dBu: boundary col0=0 (persists in buffer), col1=carry
                dbus = spool.tile([128, N * SEG], bf16, tag=f"d{db}")
                d3 = seg3(dbus)
                if j < 2:
                    nc.vector.memset(d3[:, :, 0:1], 0.0)
                if j == 0:
                    nc.vector.memset(d3[:, :, 1:2], 0.0)
                else:
                    nc.vector.tensor_copy(d3[:, :, 1:2],
                                          st_prev[db][:].rearrange("p (n o) -> p n o", o=1))
                du1 = dtuT[db][:].rearrange("p (o c) -> p o c", o=1)
                vd.tensor_tensor(out=d3[:, :, 2:SEG],
                                 in0=du1.broadcast_to([128, N, T]),
                                 in1=bc3[:, :, 0:T], op=OP.mult)
                dbs_[db] = dbus
            for db in range(NDB):
                # in-place: the scan's output stream lags its input stream,
                # so writing h over dbus (exact alias) is safe
                nc.vector.tensor_tensor_scan(dbs_[db][:], ats[db][:], dbs_[db][:],
                                             0.0, OP.mult, OP.add)
                hts.append(dbs_[db])
            for db in range(NDB):
                st = stp.tile([128, N], bf16, tag=f"st{db}")
                nc.vector.tensor_copy(st[:].rearrange("p (n o) -> p n o", o=1),
                                      seg3(hts[db])[:, :, SEG - 1:SEG])
                st_prev[db] = st
            hcs = [None] * NDB
            for db in range(NDB):
                vh = nc.gpsimd if db in HC_POOL else nc.vector
                # hc = h * C_n (token cols only), written over a_t
                vh.tensor_tensor(out=seg3(ats[db])[:, :, 2:SEG],
                                 in0=seg3(hts[db])[:, :, 2:SEG],
                                 in1=bc3[:, :, T:2 * T], op=OP.mult)
                hcs[db] = (ats[db], seg3(ats[db]), SEG, 2)
            pend = {}
            for db in range(NDB):
                vt = nc.gpsimd if db in TREE_POOL else nc.vector
                hctile, hc3, W, off = hcs[db]
                nn = N
                while nn > 2:
                    nn //= 2
                    vt.tensor_tensor(
                        out=hc3[:, 0:nn, off:W], in0=hc3[:, 0:nn, off:W],
                        in1=hc3[:, nn:2 * nn, off:W], op=OP.add)
                if db in TREE_POOL:
                    # Pool tree: defer the final add + D-term + gating to
                    # stage C so the slow Pool chain never stalls DVE here.
                    vt.tensor_tensor(out=hctile[:, off:off + T],
                                     in0=hctile[:, off:off + T],
                                     in1=hctile[:, W + off:W + off + T], op=OP.add)
                    pend[db] = (hctile, off)
                    continue
                yT = pp2.tile([128, T], bf16, tag=f"yT{db}", bufs=1)
                vt.tensor_tensor(out=yT[:], in0=hctile[:, off:off + T],
                                 in1=hctile[:, W + off:W + off + T], op=OP.add)
                # y += D * u ; gate with silu(z)
                nc.vector.scalar_tensor_tensor(
                    out=yT[:], in0=uT[db][:], scalar=mD[db][:],
                    in1=yT[:], op0=OP.mult, op1=OP.add)
                nc.vector.tensor_tensor(out=g1[db][:], in0=yT[:],
                                        in1=szT[db][:], op=OP.mult)
            return dict(g1=g1, pend=pend, uT=uT, szT=szT)

        def run_out(j, ssm, sg_j):
            """Stage C of tile j: out-proj, gating, final projection, DMA."""
            g1, pend, uT, szT = ssm["g1"], ssm["pend"], ssm["uT"], ssm["szT"]
            for db, (hctile, off) in pend.items():
                yT = pp2.tile([128, T], bf16, tag=f"yT{db}", bufs=1)
                nc.vector.scalar_tensor_tensor(
                    out=yT[:], in0=uT[db][:], scalar=mD[db][:],
                    in1=hctile[:, off:off + T], op0=OP.mult, op1=OP.add)
                nc.vector.tensor_tensor(out=g1[db][:], in0=yT[:],
                                        in1=szT[db][:], op=OP.mult)
            # ---- out-proj (+gate) ----
            moT = [pp1.tile([128, T], bf16, tag=f"moT{db}", name=f"moT{db}") for db in range(NDB)]
            for mb in range(NDB):
                pt = psum.tile([128, T], f32, tag="mm")
                for kb in range(NDB):
                    nc.tensor.matmul(pt[:], moT2[kb][:, mb * 128:(mb + 1) * 128],
                                     g1[kb][:], start=(kb == 0), stop=(kb == NDB - 1))
                mo_s = pp2.tile([128, T], bf16, tag="mo_s")
                nc.scalar.copy(mo_s[:], pt[:])
                nc.vector.tensor_tensor(out=moT[mb][:], in0=mo_s[:],
                                        in1=sg_j[mb][:], op=OP.mult)

            # ---- final projection + transpose + DMA out (from PSUM) ----
            for mb in range(DM // 128):
                pt = psum.tile([128, T], f32, tag="mm")
                for kb in range(NDB):
                    nc.tensor.matmul(pt[:], woT[kb][:, mb * 128:(mb + 1) * 128],
                                     moT[kb][:], start=(kb == 0), stop=(kb == NDB - 1))
                ot = pp2.tile([128, T], bf16, tag="ot")
                nc.scalar.copy(ot[:], pt[:])
                for tb in range(NTB):
                    pt2 = ptr.tile([128, 128], bf16, tag="tr")
                    nc.tensor.transpose(pt2[:], ot[:, tb * 128:(tb + 1) * 128], ident[:])
                    ob = pp2.tile([128, 128], bf16, tag="ob")
                    nc.scalar.copy(ob[:], pt2[:])
                    nc.sync.dma_start(
                        part[j * T + tb * 128:j * T + (tb + 1) * 128,
                             mb * 128:(mb + 1) * 128],
                        ob[:])

        # ================= main loop =================
        for i in range(nt):
            # stream in this tile's tokens: [128p, NTB blocks x DM]
            xtile = pp2.tile([128, NTB * DM], bf16, tag="xtile", bufs=1)
            nc.sync.dma_start(
                xtile[:].rearrange("p (a d) -> p a d", d=DM),
                x_in[i * T:(i + 1) * T, :].rearrange("(a p) d -> p a d", p=128))
            # LN + transpose (stats for both token blocks batched [128,2])
            hT = [pp2.tile([128, T], bf16, tag=f"hT{db}", name=f"hT{db}") for db in range(DM // 128)]
            s1 = smalls.tile([128, NTB], f32, tag="s1")
            s2 = smalls.tile([128, NTB], f32, tag="s2")
            sq = pp2.tile([128, NTB * DM], bf16, tag="sq", bufs=1)
            nc.vector.tensor_tensor(out=sq[:], in0=xtile[:], in1=xtile[:], op=OP.mult)
            for tb in range(NTB):
                nc.vector.tensor_reduce(s1[:, tb:tb + 1],
                                        xtile[:, tb * DM:(tb + 1) * DM],
                                        axis=AX.X, op=OP.add)
                nc.vector.tensor_reduce(s2[:, tb:tb + 1],
                                        sq[:, tb * DM:(tb + 1) * DM],
                                        axis=AX.X, op=OP.add)
            m = smalls.tile([128, NTB], f32, tag="m")
            nc.scalar.mul(m[:], s1[:], 1.0 / DM)
            v = smalls.tile([128, NTB], f32, tag="v")
            nc.vector.tensor_tensor(out=v[:], in0=m[:], in1=m[:], op=OP.mult)
            v2 = smalls.tile([128, NTB], f32, tag="v2")
            nc.scalar.mul(v2[:], s2[:], 1.0 / DM)
            v3 = smalls.tile([128, NTB], f32, tag="v3")
            nc.vector.tensor_tensor(out=v3[:], in0=v2[:], in1=v[:], op=OP.subtract)
            v4 = smalls.tile([128, NTB], f32, tag="v4")
            nc.vector.tensor_scalar(v4[:], v3[:], EPS, None, OP.add)
            rv = smalls.tile([128, NTB], f32, tag="rv")
            nc.vector.reciprocal(rv[:], v4[:])
            rstd = smalls.tile([128, NTB], f32, tag="rstd")
            nc.scalar.activation(rstd[:], rv[:], AF.Sqrt)
            for tb in range(NTB):
                xln = pp2.tile([128, DM], bf16, tag="xln")
                nc.vector.tensor_scalar(xln[:], xtile[:, tb * DM:(tb + 1) * DM],
                                        m[:, tb:tb + 1], rstd[:, tb:tb + 1],
                                        OP.subtract, OP.mult)
                for db in range(DM // 128):
                    pt2 = ptr.tile([128, 128], bf16, tag="tr")
                    nc.tensor.transpose(pt2[:], xln[:, db * 128:(db + 1) * 128], ident[:])
                    nc.scalar.copy(hT[db][:, tb * 128:(tb + 1) * 128], pt2[:])

            # outer in-proj -> cin (conv part) + silu(gate)
            cin = [cinp.tile([128, T + 9], bf16, tag=f"cin{db}", name=f"cin{db}") for db in range(NDB)]
            sgT = [pp2.tile([128, T], bf16, tag=f"sgT{db}", name=f"sgT{db}", bufs=4) for db in range(NDB)]
            for mb in range(2 * DH // 128):
                pt = psum.tile([128, T], f32, tag="mm")
                for kb in range(DM // 128):
                    nc.tensor.matmul(pt[:], winT[kb][:, mb * 128:(mb + 1) * 128],
                                     hT[kb][:], start=(kb == 0), stop=(kb == DM // 128 - 1))
                if mb < NDB:
                    nc.scalar.copy(cin[mb][:, 6:6 + T], pt[:])
                else:
                    db = mb - NDB
                    nc.scalar.activation(sgT[db][:], pt[:], AF.Silu, bias=gbias[db][:])

            for db in range(NDB):
                if i == 0:
                    nc.gpsimd.memset(cin[db][:, 0:6], 0.0)
                else:
                    nc.vector.tensor_copy(cin[db][:, 0:6], cin_prev[db][:, T:T + 6])
                    nc.vector.tensor_copy(cin_prev[db][:, T + 6:T + 9], cin[db][:, 6:9])

            if i > 0:
                hd_hist[i - 1] = run_head(i - 1, cin_prev)
            if i > 1:
                g1_hist[i - 2] = run_ssm(i - 2, hd_hist.pop(i - 2))
            if i > 2:
                run_out(i - 3, g1_hist.pop(i - 3), sg_hist.pop(i - 3))
            cin_prev = cin
            sg_hist[i] = sgT

        for db in range(NDB):
            nc.gpsimd.memset(cin_prev[db][:, T + 6:T + 9], 0.0)
        hd_hist[nt - 1] = run_head(nt - 1, cin_prev)
        for i in (nt - 2, nt - 1):
            g1_hist[i] = run_ssm(i, hd_hist.pop(i))
        for i in (nt - 3, nt - 2, nt - 1):
            run_out(i, g1_hist.pop(i), sg_hist.pop(i))

    nc.compile()
    return nc


def host_prepare(inputs, Lx=L):
    import ml_dtypes
    f32 = np.float32
    bf = ml_dtypes.bfloat16
    x = np.asarray(inputs["x"], f32)
    ln_g = np.asarray(inputs["ln_g"], f32)
    ln_b = np.asarray(inputs["ln_b"], f32)
    in_w = np.asarray(inputs["in_w"], f32)
    conv_w = np.asarray(inputs["conv_w"], f32)
    conv_b = np.asarray(inputs["conv_b"], f32)
    out_w = np.asarray(inputs["out_w"], f32)

    in_w_eff = in_w * ln_g[None, :]
    bias_vec = in_w @ ln_b

    core_maps, meta = [], []
    for b in range(x.shape[0]):
        for d, p in enumerate(("f", "b")):
            m_in_w = np.asarray(inputs[p + "_in_w"], f32)
            m_conv_w = np.asarray(inputs[p + "_conv_w"], f32)
            m_conv_b = np.asarray(inputs[p + "_conv_b"], f32)
            m_xproj = np.asarray(inputs[p + "_xproj_w"], f32)
            m_dt_w = np.asarray(inputs[p + "_dt_w"], f32)
            m_dt_b = np.asarray(inputs[p + "_dt_b"], f32)
            m_D = np.asarray(inputs[p + "_D"], f32)
            m_out_w = np.asarray(inputs[p + "_out_w"], f32)

            xc = x[b] if d == 0 else x[b, ::-1]
            w7 = np.zeros((DH, 7), f32)
            if d == 0:
                w7[:, 0:4] = conv_w
            else:
                w7[:, 3:7] = conv_w[:, ::-1]
            cb_eff = conv_b + bias_vec[:DH] * conv_w.sum(axis=1)
            mo2 = -m_out_w.T

            core_maps.append({
                "x_in": np.ascontiguousarray(xc).astype(bf),
                "w_in_T": np.ascontiguousarray(in_w_eff.T).astype(bf),
                "gate_bias": np.ascontiguousarray(bias_vec[DH:, None], f32),
                "w7": w7,
                "conv_b": np.ascontiguousarray(cb_eff[:, None], f32),
                "m_in_T": np.ascontiguousarray(m_in_w.T).astype(bf),
                "m_conv_w": np.ascontiguousarray(m_conv_w, f32),
                "m_conv_b": np.ascontiguousarray(m_conv_b[:, None], f32),
                "m_xproj_T": np.ascontiguousarray(m_xproj.T).astype(bf),
                "m_dt_wT": np.ascontiguousarray(m_dt_w.T).astype(bf),
                "m_dt_b": np.ascontiguousarray(-m_dt_b[:, None], f32),
                "m_out_T2": np.ascontiguousarray(mo2).astype(bf),
                "m_D": np.ascontiguousarray(-m_D[:, None], f32),
                "w_out_sl_T": np.ascontiguousarray(
                    out_w[:, d * DH:(d + 1) * DH].T).astype(bf),
            })
            meta.append((b, d))
    return core_maps, meta


def kernel(**inputs) -> np.ndarray:
    from concourse.bass_utils import run_bass_kernel_spmd

    if "nc" not in _CACHE:
        _CACHE["nc"] = build_program()
    nc = _CACHE["nc"]

    core_maps, meta = host_prepare(inputs)
    res = run_bass_kernel_spmd(nc, core_maps, list(range(len(core_maps))))
    x = np.asarray(inputs["x"], np.float32)
    out = np.array(x, np.float32, copy=True)
    for i, (b, d) in enumerate(meta):
        p = np.asarray(res.results[i]["part"], np.float32)
        out[b] += p if d == 0 else p[::-1]
    return out


# revision 36
# speedup vs baseline: 1.0117x; 1.0117x over previous
"""Bass/Trainium2 kernel for nn_NeuroBiMambaBlock.

Sharding: 8 cores = 4 samples x 2 directions (fwd/bwd mamba). Every core
runs an identical SPMD program on its own data: bwd cores receive the
time-flipped sample and the b_* weight set, so their mamba scan is
forward-in-layout. The outer (shared) stage is replicated per pair; its
causal conv becomes anti-causal on flipped cores, handled by a 7-tap
"wide" conv whose taps the host builds per direction. Each core returns
a partial output (its direction's contribution through the final
projection); the host sums the pair, un-flips the bwd part, and adds the
residual.

Layout on device: [feature -> partitions, time -> free]. The selective
scan runs as one tensor_tensor_scan per 128-channel block, with the 16
states per channel packed as 16 segments of (2 boundary cols + T token
cols); boundary col1 injects the carried state (a=0 there resets).

Engine split: depthwise convs run as diagonal matmuls on the PE; silu /
sigmoid / ln / PSUM->SBUF moves run on the Act engine; per-state decay
powers, dt*u*B and h*C products are batched into single strided
tensor ops with broadcast APs, distributed between DVE and Pool.

Four-stage software pipeline per 256-token tile — A: LN + outer in-proj
(PE/Act); B1: convs, inner proj, dt, B/C broadcast (PE/Act); B2: decay
powers, dBu, scan, h*C, tree-sum (DVE/Pool); C: out-proj + gating +
final projection + DMA. Stage k of tile i overlaps stage k-1 of tile
i+1, so Act's broadcast production always runs a full stage ahead of
the DVE consumer and the scan section pipelines cleanly.
"""

import numpy as np

B, L, DM = 4, 4096, 256
DH = 512
N = 16
KC = 4
R = 32
EPS = 1e-5
T = 256                  # tokens per pipeline tile
NT = L // T              # tiles
SEG = T + 2              # scan segment length (2 boundary cols + T tokens)
NDB = DH // 128          # 4 channel blocks
NTB = T // 128           # token blocks per tile

# engine assignment knobs (db indices whose ops run on Pool/gpsimd)
POW_POOL = frozenset()
DBU_POOL = frozenset({1, 2, 3})
HC_POOL = frozenset({2})
TREE_POOL = frozenset({2, 3})
BCALL_DVE = 2            # of 16 bcast copies, this many go to DVE (rest Act)

_CACHE = {}


def build_program(Lx=L):
    import concourse.bass as bass
    import concourse.bacc as bacc
    import concourse.tile as tile
    import concourse.mybir as mybir
    from contextlib import ExitStack

    f32 = mybir.dt.float32
    bf16 = mybir.dt.bfloat16
    AF = mybir.ActivationFunctionType
    OP = mybir.AluOpType
    AX = mybir.AxisListType

    nt = Lx // T
    nc = bacc.Bacc("TRN2", target_bir_lowering=False, debug=False)

    x_in = nc.declare_dram_parameter("x_in", [Lx, DM], bf16, isOutput=False)
    w_in_T = nc.declare_dram_parameter("w_in_T", [DM, 2 * DH], bf16, isOutput=False)
    # packed per-channel vectors [DH, 16]:
    # 0=-m_D, 1=gate_bias, 2=conv_b_eff, 3=m_conv_b, 4=-m_dt_b, 5:12=w7, 12:16=m_conv_w
    cvecd = nc.declare_dram_parameter("cvec", [DH, 16], f32, isOutput=False)
    m_in_T = nc.declare_dram_parameter("m_in_T", [DH, 2 * DH], bf16, isOutput=False)
    m_xproj_T = nc.declare_dram_parameter("m_xproj_T", [DH, R + 2 * N], bf16, isOutput=False)
    m_dt_wT = nc.declare_dram_parameter("m_dt_wT", [R, DH], bf16, isOutput=False)
    m_out_T2 = nc.declare_dram_parameter("m_out_T2", [DH, DH], bf16, isOutput=False)
    w_out_sl_T = nc.declare_dram_parameter("w_out_sl_T", [DH, DM], bf16, isOutput=False)
    part = nc.declare_dram_parameter("part", [Lx, DM], bf16, isOutput=True)

    with tile.TileContext(nc) as tc, ExitStack() as ctx:
        wpool = ctx.enter_context(tc.tile_pool(name="weights", bufs=1))
        psum = ctx.enter_context(tc.tile_pool(name="psum", bufs=2, space="PSUM"))
        pconv = ctx.enter_context(tc.tile_pool(name="pconv", bufs=2, space="PSUM"))
        pbc = ctx.enter_context(tc.tile_pool(name="pbc", bufs=2, space="PSUM"))
        ptr = ctx.enter_context(tc.tile_pool(name="ptr", bufs=2, space="PSUM"))
        pp1 = ctx.enter_context(tc.tile_pool(name="pipe1", bufs=2))
        pp2 = ctx.enter_context(tc.tile_pool(name="pipe2", bufs=2))
        cinp = ctx.enter_context(tc.tile_pool(name="cin", bufs=2))
        xinp = ctx.enter_context(tc.tile_pool(name="xin", bufs=2))
        spool = ctx.enter_context(tc.tile_pool(name="scan", bufs=1))
        bcp = ctx.enter_context(tc.tile_pool(name="bcall", bufs=1))
        stp = ctx.enter_context(tc.tile_pool(name="state", bufs=2))
        smalls = ctx.enter_context(tc.tile_pool(name="smalls", bufs=2))

        # ---- weights to SBUF (consolidated DMAs, ordered by first use) ----
        winT_t = wpool.tile([128, 2 * 2 * DH], bf16, tag="winT", name="winT")
        nc.sync.dma_start(
            winT_t[:].rearrange("p (a c) -> p a c", c=2 * DH),
            w_in_T[:].rearrange("(a p) c -> p a c", p=128))
        winT = [winT_t[:, kb * 2 * DH:(kb + 1) * 2 * DH] for kb in range(DM // 128)]

        cvt = wpool.tile([128, NDB * 16], f32, tag="cvec", name="cvec")
        nc.sync.dma_start(
            cvt[:].rearrange("p (a c) -> p a c", c=16),
            cvecd[:].rearrange("(a p) c -> p a c", p=128))
        mD = [cvt[:, db * 16 + 0:db * 16 + 1] for db in range(NDB)]
        gbias = [cvt[:, db * 16 + 1:db * 16 + 2] for db in range(NDB)]
        cbias = [cvt[:, db * 16 + 2:db * 16 + 3] for db in range(NDB)]
        mcbias = [cvt[:, db * 16 + 3:db * 16 + 4] for db in range(NDB)]
        mdtb = [cvt[:, db * 16 + 4:db * 16 + 5] for db in range(NDB)]
        w7c = [cvt[:, db * 16 + 5:db * 16 + 12] for db in range(NDB)]
        mcw = [cvt[:, db * 16 + 12:db * 16 + 16] for db in range(NDB)]

        minT_t = wpool.tile([128, NDB * 2 * DH], bf16, tag="minT", name="minT")
        nc.sync.dma_start(
            minT_t[:].rearrange("p (a c) -> p a c", c=2 * DH),
            m_in_T[:].rearrange("(a p) c -> p a c", p=128))
        minT = [minT_t[:, kb * 2 * DH:(kb + 1) * 2 * DH] for kb in range(NDB)]

        NXP = R + 2 * N
        mxpT_t = wpool.tile([128, NDB * NXP], bf16, tag="mxpT", name="mxpT")
        nc.sync.dma_start(
            mxpT_t[:].rearrange("p (a c) -> p a c", c=NXP),
            m_xproj_T[:].rearrange("(a p) c -> p a c", p=128))
        mxpT = [mxpT_t[:, kb * NXP:(kb + 1) * NXP] for kb in range(NDB)]

        mdtT = wpool.tile([R, DH], bf16)
        nc.sync.dma_start(mdtT[:], m_dt_wT[:])

        # identity for PE transposes + diag conv weights
        idf = wpool.tile([128, 128], f32)
        pidx = wpool.tile([128, 1], f32)
        nc.gpsimd.iota(idf[:], [[1, 128]], channel_multiplier=0,
                       allow_small_or_imprecise_dtypes=True)
        nc.gpsimd.iota(pidx[:], [[0, 1]], channel_multiplier=1,
                       allow_small_or_imprecise_dtypes=True)
        eqf = wpool.tile([128, 128], f32)
        nc.vector.tensor_scalar(eqf[:], idf[:], pidx[:], None, OP.is_equal)
        ident = wpool.tile([128, 128], bf16)
        nc.vector.tensor_copy(ident[:], eqf[:])
        dg_o = []          # outer conv diag taps [db][k]
        for db in range(NDB):
            row = []
            for k in range(7):
                dg = wpool.tile([128, 128], bf16, tag=f"dgo{db}_{k}", name=f"dgo{db}_{k}")
                nc.vector.tensor_scalar(dg[:], eqf[:], w7c[db][:, k:k + 1], None, OP.mult)
                row.append(dg)
            dg_o.append(row)
        dg_i = []          # inner conv diag taps [db][k]
        for db in range(NDB):
            row = []
            for k in range(KC):
                dg = wpool.tile([128, 128], bf16, tag=f"dgi{db}_{k}", name=f"dgi{db}_{k}")
                nc.vector.tensor_scalar(dg[:], eqf[:], mcw[db][:, k:k + 1], None, OP.mult)
                row.append(dg)
            dg_i.append(row)

        # selector [16, 16*128]: sel[k, n*128+m] = (k == n), for PE row-broadcast
        sel = wpool.tile([16, N * 128], bf16)
        nc.gpsimd.iota(sel[:], [[1, N], [0, 128]], channel_multiplier=0,
                       allow_small_or_imprecise_dtypes=True)
        pidx16 = wpool.tile([16, 1], f32)
        nc.gpsimd.iota(pidx16[:], [[0, 1]], channel_multiplier=1,
                       allow_small_or_imprecise_dtypes=True)
        nc.vector.tensor_scalar(sel[:], sel[:], pidx16[:], None, OP.is_equal)

        # stage-C weights, loaded last (not needed until C(0))
        moT2_t = wpool.tile([128, NDB * DH], bf16, tag="moT2", name="moT2")
        nc.sync.dma_start(
            moT2_t[:].rearrange("p (a c) -> p a c", c=DH),
            m_out_T2[:].rearrange("(a p) c -> p a c", p=128))
        moT2 = [moT2_t[:, kb * DH:(kb + 1) * DH] for kb in range(NDB)]
        woT_t = wpool.tile([128, NDB * DM], bf16, tag="woT", name="woT")
        nc.sync.dma_start(
            woT_t[:].rearrange("p (a c) -> p a c", c=DM),
            w_out_sl_T[:].rearrange("(a p) c -> p a c", p=128))
        woT = [woT_t[:, kb * DM:(kb + 1) * DM] for kb in range(NDB)]



        cin_prev = [None] * NDB
        g1_hist, sg_hist, hd_hist = {}, {}, {}
        xin_prev = [None] * NDB
        st_prev = [None] * NDB     # carried scan states [128,16] per dblk

        def seg3(ap):
            return ap[:].rearrange("p (n c) -> p n c", c=SEG)

        def run_head(j, cin_j):
            """Stage B1 of tile j: convs, projections, dt, bcall (PE+Act)."""
            # ---- outer wide conv (7 taps) on PE + silu on Act ----
            actT = [pp1.tile([128, T], bf16, tag=f"actT{db}", name=f"actT{db}") for db in range(NDB)]
            for db in range(NDB):
                pc = pconv.tile([128, T], f32, tag="cv")
                for k in range(7):
                    nc.tensor.matmul(pc[:], dg_o[db][k][:], cin_j[db][:, 3 + k:3 + k + T],
                                     start=(k == 0), stop=(k == 6))
                nc.scalar.activation(actT[db][:], pc[:], AF.Silu, bias=cbias[db][:])

            # ---- inner in-proj ----
            xin = [xinp.tile([128, T + 3], bf16, tag=f"xin{db}", name=f"xin{db}") for db in range(NDB)]
            szT = [pp1.tile([128, T], bf16, tag=f"szT{db}", name=f"szT{db}", bufs=3) for db in range(NDB)]
            zps = []
            for mb in range(2 * DH // 128):
                pt = psum.tile([128, T], f32, tag="mm")
                for kb in range(NDB):
                    nc.tensor.matmul(pt[:], minT[kb][:, mb * 128:(mb + 1) * 128],
                                     actT[kb][:], start=(kb == 0), stop=(kb == NDB - 1))
                if mb < NDB:
                    nc.scalar.copy(xin[mb][:, 3:3 + T], pt[:])
                else:
                    zps.append(pt)
            for db in range(NDB):
                if j == 0:
                    nc.gpsimd.memset(xin[db][:, 0:3], 0.0)
                else:
                    nc.vector.tensor_copy(xin[db][:, 0:3], xin_prev[db][:, T:T + 3])
                xin_prev[db] = xin[db]

            # ---- inner causal conv (4 taps) on PE + silu on Act ----
            uT = [pp1.tile([128, T], bf16, tag=f"uT{db}", name=f"uT{db}", bufs=3) for db in range(NDB)]
            for db in range(NDB):
                pc = pconv.tile([128, T], f32, tag="cv")
                for k in range(KC):
                    nc.tensor.matmul(pc[:], dg_i[db][k][:], xin[db][:, k:k + T],
                                     start=(k == 0), stop=(k == KC - 1))
                nc.scalar.activation(uT[db][:], pc[:], AF.Silu, bias=mcbias[db][:])

            # ---- xproj (dt-rank rows / combined B+C rows) ----
            pxd = psum.tile([R, T], f32, tag="mm")
            for kb in range(NDB):
                nc.tensor.matmul(pxd[:], mxpT[kb][:, 0:R], uT[kb][:],
                                 start=(kb == 0), stop=(kb == NDB - 1))
            pxb = psum.tile([N, T], f32, tag="mm")
            for kb in range(NDB):
                nc.tensor.matmul(pxb[:], mxpT[kb][:, R:R + N], uT[kb][:],
                                 start=(kb == 0), stop=(kb == NDB - 1))
            pxc = psum.tile([N, T], f32, tag="mm")
            for kb in range(NDB):
                nc.tensor.matmul(pxc[:], mxpT[kb][:, R + N:R + 2 * N], uT[kb][:],
                                 start=(kb == 0), stop=(kb == NDB - 1))
            xdbl = pp1.tile([R, T], bf16, tag="xdbl")
            nc.scalar.copy(xdbl[:], pxd[:])
            xdBC = pp1.tile([N, 2 * T], bf16, tag="xdBC")
            nc.scalar.copy(xdBC[:, 0:T], pxb[:])
            nc.scalar.copy(xdBC[:, T:2 * T], pxc[:])

            for db in range(NDB):
                nc.scalar.activation(szT[db][:], zps[db][:], AF.Silu)

            # ---- dt: rf = sigmoid(-(proj+b)) = exp(-dt); mdt = ln(rf) = -dt ----
            rT = [pp1.tile([128, T], bf16, tag=f"rT{db}", name=f"rT{db}") for db in range(NDB)]
            mdt = [pp1.tile([128, T], bf16, tag=f"mdt{db}", name=f"mdt{db}") for db in range(NDB)]
            for db in range(NDB):
                pt = psum.tile([128, T], f32, tag="mm")
                nc.tensor.matmul(pt[:], mdtT[:, db * 128:(db + 1) * 128],
                                 xdbl[:], start=True, stop=True)
                nc.scalar.activation(rT[db][:], pt[:], AF.Sigmoid, scale=-1.0,
                                     bias=mdtb[db][:])
            for db in range(NDB):
                nc.scalar.activation(mdt[db][:], rT[db][:], AF.Ln)

            # ---- broadcast B,C rows to 128 partitions: bcall[n] = [B_n | C_n] ----
            bcall = bcp.tile([128, N * 2 * T], bf16, tag="bcall")
            for n in range(N):
                pb = pbc.tile([128, 2 * T], f32, tag="bc")
                nc.tensor.matmul(pb[:], sel[:, n * 128:(n + 1) * 128], xdBC[:],
                                 start=True, stop=True)
                if n < BCALL_DVE:
                    nc.vector.tensor_copy(bcall[:, n * 2 * T:(n + 1) * 2 * T], pb[:])
                else:
                    nc.scalar.copy(bcall[:, n * 2 * T:(n + 1) * 2 * T], pb[:])
            return dict(bcall=bcall, rT=rT, mdt=mdt, uT=uT, szT=szT)

        def run_ssm(j, hd):
            """Stage B2 of tile j: powers, dBu, scan, hC, tree (DVE+Pool)."""
            bcall, rT, mdt, uT, szT = hd["bcall"], hd["rT"], hd["mdt"], hd["uT"], hd["szT"]
            bc3 = bcall[:].rearrange("p (n c) -> p n c", c=2 * T)
            dtuT = [pp1.tile([128, T], bf16, tag=f"dtuT{db}", name=f"dtuT{db}") for db in range(NDB)]
            for db in range(NDB):
                nc.vector.tensor_tensor(out=dtuT[db][:], in0=mdt[db][:],
                                        in1=uT[db][:], op=OP.mult)

            # ---- per channel block: powers, dBu, scan, hC, tree ----
            # Emitted stage-parallel (all dbs per stage) so independent db
            # work sits adjacent in each engine queue and overlaps.
            g1 = [pp1.tile([128, T], bf16, tag=f"g1{db}", name=f"g1{db}") for db in range(NDB)]
            ats, hts = [], []
            for db in range(NDB):
                vp = nc.gpsimd if db in POW_POOL else nc.vector
                # a_t: seg n tokens = r^(n+1); boundary cols 0 (via propagation)
                a_t = spool.tile([128, N * SEG], bf16, tag=f"a{db}",
                                 bufs=2 if db in TREE_POOL else 1)
                a3 = seg3(a_t)
                if j < 2:
                    nc.vector.memset(a3[:, 0:2, 0:2], 0.0)
                nc.vector.tensor_copy(a_t[:, 2:2 + T], rT[db][:])
                vp.tensor_tensor(out=a_t[:, SEG + 2:SEG + 2 + T], in0=rT[db][:],
                                 in1=rT[db][:], op=OP.mult)
                for (lo, hi) in ((2, 4), (4, 8), (8, 16)):
                    seg_hi = a_t[:, (lo - 1) * SEG:lo * SEG].rearrange(
                        "p (o c) -> p o c", o=1)
                    vp.tensor_tensor(out=a3[:, lo:hi, :], in0=a3[:, 0:hi - lo, :],
                                     in1=seg_hi.broadcast_to([128, hi - lo, SEG]),
                                     op=OP.mult)
                ats.append(a_t)
            dbs_ = [None] * NDB
            for db in sorted(range(NDB), key=lambda d: (d not in DBU_POOL, d)):
                vd = nc.gpsimd if db in DBU_POOL else nc.vector
                # dBu: boundary col0=0 (persists in buffer), col1=carry
                dbus = spool.tile([128, N * SEG], bf16, tag=f"d{db}")
                d3 = seg3(dbus)
                if j < 2:
                    nc.vector.memset(d3[:, :, 0:1], 0.0)
                if j == 0:
                    nc.vector.memset(d3[:, :, 1:2], 0.0)
                else:
                    nc.vector.tensor_copy(d3[:, :, 1:2],
                                          st_prev[db][:].rearrange("p (n o) -> p n o", o=1))
                du1 = dtuT[db][:].rearrange("p (o c) -> p o c", o=1)
                vd.tensor_tensor(out=d3[:, :, 2:SEG],
                                 in0=du1.broadcast_to([128, N, T]),
                                 in1=bc3[:, :, 0:T], op=OP.mult)
                dbs_[db] = dbus
            for db in range(NDB):
                # in-place: the scan's output stream lags its input stream,
                # so writing h over dbus (exact alias) is safe
                nc.vector.tensor_tensor_scan(dbs_[db][:], ats[db][:], dbs_[db][:],
                                             0.0, OP.mult, OP.add)
                hts.append(dbs_[db])
            for db in range(NDB):
                st = stp.tile([128, N], bf16, tag=f"st{db}")
                nc.vector.tensor_copy(st[:].rearrange("p (n o) -> p n o", o=1),
                                      seg3(hts[db])[:, :, SEG - 1:SEG])
                st_prev[db] = st
            hcs = [None] * NDB
            for db in range(NDB):
                vh = nc.gpsimd if db in HC_POOL else nc.vector
                # hc = h * C_n (token cols only), written over a_t
                vh.tensor_tensor(out=seg3(ats[db])[:, :, 2:SEG],
                                 in0=seg3(hts[db])[:, :, 2:SEG],
                                 in1=bc3[:, :, T:2 * T], op=OP.mult)
                hcs[db] = (ats[db], seg3(ats[db]), SEG, 2)
            pend = {}
            for db in range(NDB):
                vt = nc.gpsimd if db in TREE_POOL else nc.vector
                hctile, hc3, W, off = hcs[db]
                nn = N
                while nn > 2:
                    nn //= 2
                    vt.tensor_tensor(
                        out=hc3[:, 0:nn, off:W], in0=hc3[:, 0:nn, off:W],
                        in1=hc3[:, nn:2 * nn, off:W], op=OP.add)
                if db in TREE_POOL:
                    # Pool tree: defer the final add + D-term + gating to
                    # stage C so the slow Pool chain never stalls DVE here.
                    vt.tensor_tensor(out=hctile[:, off:off + T],
                                     in0=hctile[:, off:off + T],
                                     in1=hctile[:, W + off:W + off + T], op=OP.add)
                    pend[db] = (hctile, off)
                    continue
                yT = pp2.tile([128, T], bf16, tag=f"yT{db}", bufs=1)
                vt.tensor_tensor(out=yT[:], in0=hctile[:, off:off + T],
                                 in1=hctile[:, W + off:W + off + T], op=OP.add)
                # y += D * u ; gate with silu(z)
                nc.vector.scalar_tensor_tensor(
                    out=yT[:], in0=uT[db][:], scalar=mD[db][:],
                    in1=yT[:], op0=OP.mult, op1=OP.add)
                nc.vector.tensor_tensor(out=g1[db][:], in0=yT[:],
                                        in1=szT[db][:], op=OP.mult)
            return dict(g1=g1, pend=pend, uT=uT, szT=szT)

        def run_out(j, ssm, sg_j):
            """Stage C of tile j: out-proj, gating, final projection, DMA."""
            g1, pend, uT, szT = ssm["g1"], ssm["pend"], ssm["uT"], ssm["szT"]
            for db, (hctile, off) in pend.items():
                yT = pp2.tile([128, T], bf16, tag=f"yT{db}", bufs=1)
                nc.vector.scalar_tensor_tensor(
                    out=yT[:], in0=uT[db][:], scalar=mD[db][:],
                    in1=hctile[:, off:off + T], op0=OP.mult, op1=OP.add)
                nc.vector.tensor_tensor(out=g1[db][:], in0=yT[:],
                                        in1=szT[db][:], op=OP.mult)
            # ---- out-proj (+gate) ----
            moT = [pp1.tile([128, T], bf16, tag=f"moT{db}", name=f"moT{db}") for db in range(NDB)]
            for mb in range(NDB):
                pt = psum.tile([128, T], f32, tag="mm")
                for kb in range(NDB):
                    nc.tensor.matmul(pt[:], moT2[kb][:, mb * 128:(mb + 1) * 128],
                                     g1[kb][:], start=(kb == 0), stop=(kb == NDB - 1))
                mo_s = pp2.tile([128, T], bf16, tag="mo_s")
                nc.scalar.copy(mo_s[:], pt[:])
                nc.vector.tensor_tensor(out=moT[mb][:], in0=mo_s[:],
                                        in1=sg_j[mb][:], op=OP.mult)

            # ---- final projection + transpose + DMA out (from PSUM) ----
            for mb in range(DM // 128):
                pt = psum.tile([128, T], f32, tag="mm")
                for kb in range(NDB):
                    nc.tensor.matmul(pt[:], woT[kb][:, mb * 128:(mb + 1) * 128],
                                     moT[kb][:], start=(kb == 0), stop=(kb == NDB - 1))
                ot = pp2.tile([128, T], bf16, tag="ot")
                nc.scalar.copy(ot[:], pt[:])
                for tb in range(NTB):
                    pt2 = ptr.tile([128, 128], bf16, tag="tr")
                    nc.tensor.transpose(pt2[:], ot[:, tb * 128:(tb + 1) * 128], ident[:])
                    ob = pp2.tile([128, 128], bf16, tag="ob")
                    nc.scalar.copy(ob[:], pt2[:])
                    nc.sync.dma_start(
                        part[j * T + tb * 128:j * T + (tb + 1) * 128,
                             mb * 128:(mb + 1) * 128],
                        ob[:])

        # ================= main loop =================
        for i in range(nt):
            # stream in this tile's tokens: [128p, NTB blocks x DM]
            xtile = pp2.tile([128, NTB * DM], bf16, tag="xtile", bufs=1)
            nc.gpsimd.dma_start(
                xtile[:].rearrange("p (a d) -> p a d", d=DM),
                x_in[i * T:(i + 1) * T, :].rearrange("(a p) d -> p a d", p=128))
            # LN + transpose (stats for both token blocks batched [128,2])
            hT = [pp2.tile([128, T], bf16, tag=f"hT{db}", name=f"hT{db}") for db in range(DM // 128)]
            s1 = smalls.tile([128, NTB], f32, tag="s1")
            s2 = smalls.tile([128, NTB], f32, tag="s2")
            sq = pp2.tile([128, NTB * DM], bf16, tag="sq", bufs=1)
            nc.vector.tensor_tensor(out=sq[:], in0=xtile[:], in1=xtile[:], op=OP.mult)
            for tb in range(NTB):
                nc.vector.tensor_reduce(s1[:, tb:tb + 1],
                                        xtile[:, tb * DM:(tb + 1) * DM],
                                        axis=AX.X, op=OP.add)
                nc.vector.tensor_reduce(s2[:, tb:tb + 1],
                                        sq[:, tb * DM:(tb + 1) * DM],
                                        axis=AX.X, op=OP.add)
            m = smalls.tile([128, NTB], f32, tag="m")
            nc.scalar.mul(m[:], s1[:], 1.0 / DM)
            v = smalls.tile([128, NTB], f32, tag="v")
            nc.vector.tensor_tensor(out=v[:], in0=m[:], in1=m[:], op=OP.mult)
            v2 = smalls.tile([128, NTB], f32, tag="v2")
            nc.scalar.mul(v2[:], s2[:], 1.0 / DM)
            v3 = smalls.tile([128, NTB], f32, tag="v3")
            nc.vector.tensor_tensor(out=v3[:], in0=v2[:], in1=v[:], op=OP.subtract)
            v4 = smalls.tile([128, NTB], f32, tag="v4")
            nc.vector.tensor_scalar(v4[:], v3[:], EPS, None, OP.add)
            rv = smalls.tile([128, NTB], f32, tag="rv")
            nc.vector.reciprocal(rv[:], v4[:])
            rstd = smalls.tile([128, NTB], f32, tag="rstd")
            nc.scalar.activation(rstd[:], rv[:], AF.Sqrt)
            for tb in range(NTB):
                xln = pp2.tile([128, DM], bf16, tag="xln")
                nc.vector.tensor_scalar(xln[:], xtile[:, tb * DM:(tb + 1) * DM],
                                        m[:, tb:tb + 1], rstd[:, tb:tb + 1],
                                        OP.subtract, OP.mult)
                for db in range(DM // 128):
                    pt2 = ptr.tile([128, 128], bf16, tag="tr")
                    nc.tensor.transpose(pt2[:], xln[:, db * 128:(db + 1) * 128], ident[:])
                    nc.scalar.copy(hT[db][:, tb * 128:(tb + 1) * 128], pt2[:])

            # outer in-proj -> cin (conv part) + silu(gate)
            cin = [cinp.tile([128, T + 9], bf16, tag=f"cin{db}", name=f"cin{db}") for db in range(NDB)]
            sgT = [pp2.tile([128, T], bf16, tag=f"sgT{db}", name=f"sgT{db}", bufs=4) for db in range(NDB)]
            for mb in range(2 * DH // 128):
                pt = psum.tile([128, T], f32, tag="mm")
                for kb in range(DM // 128):
                    nc.tensor.matmul(pt[:], winT[kb][:, mb * 128:(mb + 1) * 128],
                                     hT[kb][:], start=(kb == 0), stop=(kb == DM // 128 - 1))
                if mb < NDB:
                    nc.scalar.copy(cin[mb][:, 6:6 + T], pt[:])
                else:
                    db = mb - NDB
                    nc.scalar.activation(sgT[db][:], pt[:], AF.Silu, bias=gbias[db][:])

            for db in range(NDB):
                if i == 0:
                    nc.gpsimd.memset(cin[db][:, 0:6], 0.0)
                else:
                    nc.vector.tensor_copy(cin[db][:, 0:6], cin_prev[db][:, T:T + 6])
                    nc.vector.tensor_copy(cin_prev[db][:, T + 6:T + 9], cin[db][:, 6:9])

            if i > 0:
                hd_hist[i - 1] = run_head(i - 1, cin_prev)
            if i > 1:
                g1_hist[i - 2] = run_ssm(i - 2, hd_hist.pop(i - 2))
            if i > 2:
                run_out(i - 3, g1_hist.pop(i - 3), sg_hist.pop(i - 3))
            cin_prev = cin
            sg_hist[i] = sgT

        for db in range(NDB):
            nc.gpsimd.memset(cin_prev[db][:, T + 6:T + 9], 0.0)
        hd_hist[nt - 1] = run_head(nt - 1, cin_prev)
        for i in (nt - 2, nt - 1):
            g1_hist[i] = run_ssm(i, hd_hist.pop(i))
        for i in (nt - 3, nt - 2, nt - 1):
            run_out(i, g1_hist.pop(i), sg_hist.pop(i))

    nc.compile()
    return nc


def host_prepare(inputs, Lx=L):
    import ml_dtypes
    f32 = np.float32
    bf = ml_dtypes.bfloat16
    x = np.asarray(inputs["x"], f32)
    ln_g = np.asarray(inputs["ln_g"], f32)
    ln_b = np.asarray(inputs["ln_b"], f32)
    in_w = np.asarray(inputs["in_w"], f32)
    conv_w = np.asarray(inputs["conv_w"], f32)
    conv_b = np.asarray(inputs["conv_b"], f32)
    out_w = np.asarray(inputs["out_w"], f32)

    in_w_eff = in_w * ln_g[None, :]
    bias_vec = in_w @ ln_b

    core_maps, meta = [], []
    for b in range(x.shape[0]):
        for d, p in enumerate(("f", "b")):
            m_in_w = np.asarray(inputs[p + "_in_w"], f32)
            m_conv_w = np.asarray(inputs[p + "_conv_w"], f32)
            m_conv_b = np.asarray(inputs[p + "_conv_b"], f32)
            m_xproj = np.asarray(inputs[p + "_xproj_w"], f32)
            m_dt_w = np.asarray(inputs[p + "_dt_w"], f32)
            m_dt_b = np.asarray(inputs[p + "_dt_b"], f32)
            m_D = np.asarray(inputs[p + "_D"], f32)
            m_out_w = np.asarray(inputs[p + "_out_w"], f32)

            xc = x[b] if d == 0 else x[b, ::-1]
            w7 = np.zeros((DH, 7), f32)
            if d == 0:
                w7[:, 0:4] = conv_w
            else:
                w7[:, 3:7] = conv_w[:, ::-1]
            cb_eff = conv_b + bias_vec[:DH] * conv_w.sum(axis=1)
            mo2 = -m_out_w.T

            cvec = np.zeros((DH, 16), f32)
            cvec[:, 0] = -m_D
            cvec[:, 1] = bias_vec[DH:]
            cvec[:, 2] = cb_eff
            cvec[:, 3] = m_conv_b
            cvec[:, 4] = -m_dt_b
            cvec[:, 5:12] = w7
            cvec[:, 12:16] = m_conv_w
            core_maps.append({
                "x_in": np.ascontiguousarray(xc).astype(bf),
                "w_in_T": np.ascontiguousarray(in_w_eff.T).astype(bf),
                "cvec": cvec,
                "m_in_T": np.ascontiguousarray(m_in_w.T).astype(bf),
                "m_xproj_T": np.ascontiguousarray(m_xproj.T).astype(bf),
                "m_dt_wT": np.ascontiguousarray(m_dt_w.T).astype(bf),
                "m_out_T2": np.ascontiguousarray(mo2).astype(bf),
                "w_out_sl_T": np.ascontiguousarray(
                    out_w[:, d * DH:(d + 1) * DH].T).astype(bf),
            })
            meta.append((b, d))
    return core_maps, meta


def kernel(**inputs) -> np.ndarray:
    from concourse.bass_utils import run_bass_kernel_spmd

    if "nc" not in _CACHE:
        _CACHE["nc"] = build_program()
    nc = _CACHE["nc"]

    core_maps, meta = host_prepare(inputs)
    res = run_bass_kernel_spmd(nc, core_maps, list(range(len(core_maps))))
    x = np.asarray(inputs["x"], np.float32)
    out = np.array(x, np.float32, copy=True)
    for i, (b, d) in enumerate(meta):
        p = np.asarray(res.results[i]["part"], np.float32)
        out[b] += p if d == 0 else p[::-1]
    return out
